# revision 4
# baseline (speedup 1.0000x reference)
"""Trainium2 Bass kernel for the GAT-style transformer layer (nn_GTLayer).

Math used (exact restructuring of the reference):
  score[b,h,i,j] = sl[b,h,i] + sr[b,h,j]  with
      sl = leaky(h@W_l) . a_l (per head),  sr = leaky(h@W_r) . a_r
  - softmax_j(score) = softmax_j(sr[b,h,:]) : independent of i  (shift
    invariance), so context[b,h,i,:] = c[b,h,:] = sum_j w_j fr[b,h,j,:]
    for every i, and fh = (concat_h c) @ W_final is one row per batch.
  - attn = mean_h score = sl_bar[b,i] + sr_bar[b,j]  (rank-1 outer sum).

So the kernel is memory-bound on writing attn [B,N,N] (134 MB fp32).

Two SPMD launches over 8 cores (rows of (B*N) sharded, 1024 rows/core,
each core's rows inside one batch):
  stage 1: per-core fl/fr projections (PE matmuls), leaky, per-head
           sr + local softmax partials (max / sum-exp / weighted fr sum),
           and the head-mean scores sl_bar / sr_bar for the core's rows.
  host:    combine ~20 KB of per-core softmax partials exactly
           (log-sum-exp style), tiny c @ W_final ([2,256]@[256,256]).
  stage 2: stream attn rows = sl_bar[i] + sr_bar[j] (outer sum, split
           across Vector+Scalar engines) and h_out = LN(h + fh) rows.
"""

import os
import sys
from contextlib import ExitStack

import numpy as np

for _p in ("/opt/trn_rl_repo",):
    if os.path.isdir(_p) and _p not in sys.path:
        sys.path.append(_p)

import concourse.bass as bass
import concourse.tile as tile
from concourse import bacc, mybir
from concourse.bass_utils import run_bass_kernel_spmd
from concourse.masks import make_identity

B, N, D, H, HD = 2, 4096, 256, 8, 32
NEG_SLOPE = 0.01
LN_EPS = 1e-5
NCORES = 8
RPC = B * N // NCORES  # 1024 rows per core
RT = RPC // 128        # 8 row tiles per core
KT = D // 128          # 2 contraction tiles
F32 = mybir.dt.float32
AF = mybir.ActivationFunctionType
ALU = mybir.AluOpType
AX = mybir.AxisListType

_CACHE: dict = {}
last_results: list = []  # BassKernelResults of the most recent kernel() call


def _trace_args(tag):
    d = os.environ.get("GT_TRACE_DIR")
    if not d:
        return {}
    td = os.path.join(d, tag)
    os.makedirs(td, exist_ok=True)
    return {"trace": True, "tmpdir": td}


def _build_stage1():
    nc = bacc.Bacc("TRN2", target_bir_lowering=False, debug=False, num_devices=NCORES)
    hT = nc.dram_tensor("hT", [D, RPC], F32, kind="ExternalInput").ap()
    wl = nc.dram_tensor("wl", [D, D], F32, kind="ExternalInput").ap()
    wr = nc.dram_tensor("wr", [D, D], F32, kind="ExternalInput").ap()
    alsel = nc.dram_tensor("alsel", [D, 1], F32, kind="ExternalInput").ap()
    arsel = nc.dram_tensor("arsel", [D, H + 1], F32, kind="ExternalInput").ap()
    slb = nc.dram_tensor("slb", [RPC], F32, kind="ExternalOutput").ap()
    srb = nc.dram_tensor("srb", [RPC], F32, kind="ExternalOutput").ap()
    mloc = nc.dram_tensor("mloc", [H], F32, kind="ExternalOutput").ap()
    sloc = nc.dram_tensor("sloc", [H], F32, kind="ExternalOutput").ap()
    csum = nc.dram_tensor("csum", [H, D], F32, kind="ExternalOutput").ap()

    with tile.TileContext(nc) as tc, ExitStack() as ctx:
        const = ctx.enter_context(tc.tile_pool(name="const", bufs=1))
        work = ctx.enter_context(tc.tile_pool(name="work", bufs=2))
        ps = ctx.enter_context(tc.tile_pool(name="ps", bufs=2, space="PSUM"))
        ps1 = ctx.enter_context(tc.tile_pool(name="ps1", bufs=1, space="PSUM"))

        sb_hT = []
        sb_wl = []
        sb_wr = []
        sb_al = []
        sb_ar = []
        for k in range(KT):
            t = const.tile([128, RPC], F32, tag=f"hT{k}")
            nc.sync.dma_start(t[:], hT[k * 128:(k + 1) * 128, :])
            sb_hT.append(t)
            t = const.tile([128, D], F32, tag=f"wl{k}")
            nc.sync.dma_start(t[:], wl[k * 128:(k + 1) * 128, :])
            sb_wl.append(t)
            t = const.tile([128, D], F32, tag=f"wr{k}")
            nc.sync.dma_start(t[:], wr[k * 128:(k + 1) * 128, :])
            sb_wr.append(t)
            t = const.tile([128, 1], F32, tag=f"al{k}")
            nc.sync.dma_start(t[:], alsel[k * 128:(k + 1) * 128, :])
            sb_al.append(t)
            t = const.tile([128, H + 1], F32, tag=f"ar{k}")
            nc.sync.dma_start(t[:], arsel[k * 128:(k + 1) * 128, :])
            sb_ar.append(t)
        ident = const.tile([H, H], F32, tag="ident")
        make_identity(nc, ident[:])

        # leaky projections in [dout, row] layout: lf*[m][d', j]
        def leaky_proj(w_tiles, name):
            out = []
            for m in range(KT):
                t = const.tile([128, RPC], F32, tag=f"{name}{m}")
                out.append(t)
                for j in range(RPC // 512):
                    pt = ps.tile([128, 512], F32, tag="mm")
                    for k in range(KT):
                        nc.tensor.matmul(
                            pt[:],
                            w_tiles[k][:, m * 128:(m + 1) * 128],
                            sb_hT[k][:, j * 512:(j + 1) * 512],
                            start=(k == 0),
                            stop=(k == KT - 1),
                        )
                    # leaky(x) = 0.01*x + relu(0.99*x) ; scale commutes with relu
                    rl = work.tile([128, 512], F32, tag="rl")
                    nc.scalar.activation(rl[:], pt[:], AF.Relu, scale=1.0 - NEG_SLOPE)
                    nc.vector.scalar_tensor_tensor(
                        out=t[:, j * 512:(j + 1) * 512],
                        in0=pt[:],
                        scalar=NEG_SLOPE,
                        in1=rl[:],
                        op0=ALU.mult,
                        op1=ALU.add,
                    )
            return out

        lfl = leaky_proj(sb_wl, "lfl")
        lfr = leaky_proj(sb_wr, "lfr")

        # raw fr in [row, dout] layout for the context sum
        sb_frr = []
        for jt in range(RT):
            pt = ps.tile([128, D], F32, tag="frr")
            for k in range(KT):
                nc.tensor.matmul(
                    pt[:],
                    sb_hT[k][:, jt * 128:(jt + 1) * 128],
                    sb_wr[k][:],
                    start=(k == 0),
                    stop=(k == KT - 1),
                )
            t = work.tile([128, D], F32, tag=f"frr{jt}")
            nc.vector.tensor_copy(t[:], pt[:])
            sb_frr.append(t)

        # sl_bar[rows] = sum_d alsel[d] * lfl[d, rows]
        sb_sl = work.tile([1, RPC], F32, tag="slv")
        for j in range(RPC // 512):
            pt = ps1.tile([1, 512], F32, tag="sl")
            for k in range(KT):
                nc.tensor.matmul(
                    pt[:], sb_al[k][:], lfl[k][:, j * 512:(j + 1) * 512],
                    start=(k == 0), stop=(k == KT - 1),
                )
            nc.vector.tensor_copy(sb_sl[:, j * 512:(j + 1) * 512], pt[:])
        nc.sync.dma_start(slb[None, :], sb_sl[:])

        # sr9[h, rows]: rows 0..7 per-head scores, row 8 = head-mean
        sb_sr9 = work.tile([H + 1, RPC], F32, tag="sr9")
        for j in range(RPC // 512):
            pt = ps1.tile([H + 1, 512], F32, tag="sr")
            for k in range(KT):
                nc.tensor.matmul(
                    pt[:], sb_ar[k][:], lfr[k][:, j * 512:(j + 1) * 512],
                    start=(k == 0), stop=(k == KT - 1),
                )
            nc.vector.tensor_copy(sb_sr9[:, j * 512:(j + 1) * 512], pt[:])
        nc.sync.dma_start(srb[None, :], sb_sr9[H:H + 1, :])

        # local softmax partials over this core's rows
        sb_m = work.tile([H, 1], F32, tag="m")
        nc.vector.reduce_max(out=sb_m[:], in_=sb_sr9[0:H, :], axis=AX.X)
        sb_mneg = work.tile([H, 1], F32, tag="mneg")
        nc.scalar.mul(sb_mneg[:], sb_m[:], -1.0)
        sb_e = work.tile([H, RPC], F32, tag="e")
        nc.scalar.activation(sb_e[:], sb_sr9[0:H, :], AF.Exp, bias=sb_mneg[:])
        sb_s = work.tile([H, 1], F32, tag="s")
        nc.vector.reduce_sum(out=sb_s[:], in_=sb_e[:], axis=AX.X)
        nc.sync.dma_start(mloc[:, None], sb_m[:])
        nc.sync.dma_start(sloc[:, None], sb_s[:])

        # csum[h, d] = sum_j e[h, j] * fr[j, d]
        pc = ps1.tile([H, D], F32, tag="csum")
        sb_et = []
        for jt in range(RT):
            pe = ps1.tile([128, H], F32, tag="et")
            nc.tensor.transpose(pe[:], sb_e[:, jt * 128:(jt + 1) * 128], ident[:])
            t = work.tile([128, H], F32, tag=f"et{jt}")
            nc.vector.tensor_copy(t[:], pe[:])
            sb_et.append(t)
        for jt in range(RT):
            nc.tensor.matmul(
                pc[:], sb_et[jt][:], sb_frr[jt][:],
                start=(jt == 0), stop=(jt == RT - 1),
            )
        sb_c = work.tile([H, D], F32, tag="c")
        nc.vector.tensor_copy(sb_c[:], pc[:])
        nc.sync.dma_start(csum[:], sb_c[:])

    nc.compile()
    return nc


def _build_stage2():
    nc = bacc.Bacc("TRN2", target_bir_lowering=False, debug=False, num_devices=NCORES)
    hrows = nc.dram_tensor("hrows", [RPC, D], F32, kind="ExternalInput").ap()
    slcol = nc.dram_tensor("slcol", [128, RT], F32, kind="ExternalInput").ap()
    srbv = nc.dram_tensor("srbv", [N], F32, kind="ExternalInput").ap()
    fhv = nc.dram_tensor("fhv", [D], F32, kind="ExternalInput").ap()
    lng = nc.dram_tensor("lng", [D], F32, kind="ExternalInput").ap()
    lnb = nc.dram_tensor("lnb", [D], F32, kind="ExternalInput").ap()
    attn = nc.dram_tensor("attn", [RPC, N], F32, kind="ExternalOutput").ap()
    hout = nc.dram_tensor("hout", [RPC, D], F32, kind="ExternalOutput").ap()

    def bcast(ap_1d, n):
        return bass.AP(tensor=ap_1d.tensor, offset=ap_1d.offset, ap=[[0, 128], [1, n]])

    with tile.TileContext(nc) as tc, ExitStack() as ctx:
        const = ctx.enter_context(tc.tile_pool(name="const", bufs=1))
        apool = ctx.enter_context(tc.tile_pool(name="apool", bufs=3))
        hpool = ctx.enter_context(tc.tile_pool(name="hpool", bufs=3))
        spool = ctx.enter_context(tc.tile_pool(name="spool", bufs=4))

        sb_sr = const.tile([128, N], F32, tag="sr")
        nc.sync.dma_start(sb_sr[:], bcast(srbv, N))
        sb_fh = const.tile([128, D], F32, tag="fh")
        nc.sync.dma_start(sb_fh[:], bcast(fhv, D))
        sb_g = const.tile([128, D], F32, tag="g")
        nc.sync.dma_start(sb_g[:], bcast(lng, D))
        sb_b = const.tile([128, D], F32, tag="b")
        nc.sync.dma_start(sb_b[:], bcast(lnb, D))
        sb_sl = const.tile([128, RT], F32, tag="sl")
        nc.sync.dma_start(sb_sl[:], slcol[:])
        sb_eps = const.tile([128, 1], F32, tag="eps")
        nc.vector.memset(sb_eps[:], LN_EPS)

        for t in range(RT):
            rs_ = slice(t * 128, (t + 1) * 128)
            at = apool.tile([128, N], F32, tag="at")
            # outer sum: split halves across Vector and Scalar engines
            nc.vector.tensor_scalar_add(at[:, 0:N // 2], sb_sr[:, 0:N // 2], sb_sl[:, t:t + 1])
            nc.scalar.activation(at[:, N // 2:N], sb_sr[:, N // 2:N], AF.Identity, bias=sb_sl[:, t:t + 1])
            nc.sync.dma_start(attn[rs_, :], at[:])

            ht = hpool.tile([128, D], F32, tag="ht")
            nc.sync.dma_start(ht[:], hrows[rs_, :])
            xt = hpool.tile([128, D], F32, tag="xt")
            nc.vector.tensor_add(xt[:], ht[:], sb_fh[:])
            st = spool.tile([128, 6], F32, tag="st")
            nc.vector.bn_stats(out=st[:], in_=xt[:])
            mv = spool.tile([128, 2], F32, tag="mv")
            nc.vector.bn_aggr(out=mv[:], in_=st[:])
            rstd = spool.tile([128, 1], F32, tag="rstd")
            nc.scalar.activation(rstd[:], mv[:, 1:2], AF.Sqrt, bias=sb_eps[:])
            nc.vector.reciprocal(rstd[:], rstd[:])
            yt = hpool.tile([128, D], F32, tag="yt")
            nc.vector.tensor_scalar(
                out=yt[:], in0=xt[:],
                scalar1=mv[:, 0:1], scalar2=rstd[:],
                op0=ALU.subtract, op1=ALU.mult,
            )
            nc.vector.tensor_mul(yt[:], yt[:], sb_g[:])
            nc.vector.tensor_add(yt[:], yt[:], sb_b[:])
            nc.sync.dma_start(hout[rs_, :], yt[:])

    nc.compile()
    return nc


def _get_programs():
    if "s1" not in _CACHE:
        _CACHE["s1"] = _build_stage1()
        _CACHE["s2"] = _build_stage2()
    return _CACHE["s1"], _CACHE["s2"]


def kernel(h, W_l, W_r, a_l, a_r, W_final, ln_g, ln_b):
    global last_results
    h = np.ascontiguousarray(np.asarray(h, np.float32))
    W_l = np.asarray(W_l, np.float32)
    W_r = np.asarray(W_r, np.float32)
    a_l = np.asarray(a_l, np.float32)
    a_r = np.asarray(a_r, np.float32)
    W_final = np.asarray(W_final, np.float32)
    ln_g = np.asarray(ln_g, np.float32)
    ln_b = np.asarray(ln_b, np.float32)

    nc1, nc2 = _get_programs()
    hf = h.reshape(B * N, D)

    alsel = (np.tile(a_l, H) / H).reshape(D, 1).astype(np.float32)
    arsel = np.zeros((D, H + 1), np.float32)
    for hh in range(H):
        arsel[32 * hh:32 * hh + 32, hh] = a_r
    arsel[:, H] = np.tile(a_r, H) / H

    in1 = []
    for c in range(NCORES):
        rows = hf[c * RPC:(c + 1) * RPC]
        in1.append({
            "hT": np.ascontiguousarray(rows.T),
            "wl": W_l, "wr": W_r, "alsel": alsel, "arsel": arsel,
        })
    res1 = run_bass_kernel_spmd(nc1, in1, list(range(NCORES)), **_trace_args("s1"))
    r1 = res1.results

    slb = np.concatenate([r1[c]["slb"] for c in range(NCORES)])
    srbf = np.concatenate([r1[c]["srb"] for c in range(NCORES)])
    mloc = np.stack([r1[c]["mloc"] for c in range(NCORES)])   # [8, H]
    sloc = np.stack([r1[c]["sloc"] for c in range(NCORES)])   # [8, H]
    csum = np.stack([r1[c]["csum"] for c in range(NCORES)])   # [8, H, D]

    # exact global softmax combine + (degenerate, one row per batch) W_final matmul
    fh = np.zeros((B, D), np.float32)
    for b in range(B):
        cs = slice(b * (NCORES // B), (b + 1) * (NCORES // B))
        M = mloc[cs].max(axis=0)
        sc = np.exp(mloc[cs] - M[None, :])
        S = (sloc[cs] * sc).sum(axis=0)
        Cs = (csum[cs] * sc[:, :, None]).sum(axis=0)          # [H, D]
        cvec = np.stack([Cs[hh, 32 * hh:32 * hh + 32] / S[hh] for hh in range(H)])
        fh[b] = cvec.reshape(D).astype(np.float32) @ W_final

    sl_bar = slb.reshape(B, N)
    sr_bar = srbf.reshape(B, N)

    in2 = []
    for c in range(NCORES):
        b = c // (NCORES // B)
        sl_own = slb[c * RPC:(c + 1) * RPC]
        in2.append({
            "hrows": hf[c * RPC:(c + 1) * RPC],
            "slcol": np.ascontiguousarray(sl_own.reshape(RT, 128).T),
            "srbv": sr_bar[b],
            "fhv": fh[b],
            "lng": ln_g,
            "lnb": ln_b,
        })
    res2 = run_bass_kernel_spmd(nc2, in2, list(range(NCORES)), **_trace_args("s2"))
    r2 = res2.results
    last_results = [res1, res2]

    attn = np.concatenate([r2[c]["attn"] for c in range(NCORES)]).reshape(B, N, N)
    hout = np.concatenate([r2[c]["hout"] for c in range(NCORES)]).reshape(B, N, D)
    return hout, attn


# revision 7
# speedup vs baseline: 1.0118x; 1.0118x over previous
"""Trainium2 Bass kernel for the GAT-style transformer layer (nn_GTLayer).

Math used (exact restructuring of the reference):
  score[b,h,i,j] = sl[b,h,i] + sr[b,h,j]  with
      sl = leaky(h@W_l) . a_l (per head),  sr = leaky(h@W_r) . a_r
  - softmax_j(score) = softmax_j(sr[b,h,:]) : independent of i  (shift
    invariance), so context[b,h,i,:] = c[b,h,:] = sum_j w_j fr[b,h,j,:]
    for every i, and fh = (concat_h c) @ W_final is one row per batch.
  - attn = mean_h score = sl_bar[b,i] + sr_bar[b,j]  (rank-1 outer sum).

So the kernel is memory-bound on writing attn [B,N,N] (134 MB fp32).

Two SPMD launches over 8 cores (rows of (B*N) sharded, 1024 rows/core,
each core's rows inside one batch):
  stage 1: per-core fl/fr projections in [row, dout] layout (one fused
           [W_l|W_r] moving operand per k-tile), leaky + score dots on
           Vector, softmax partials (sum-exp and exp-weighted fr sum —
           scores are O(1) so no max shift is needed) via PE contractions.
  host:    combine ~18 KB of per-core softmax partials, tiny c @ W_final.
  stage 2: stream attn rows = sl_bar[i] + sr_bar[j] (outer sum, halves
           split across Vector+Scalar engines) and h_out = LN(h+fh) rows.
"""

import os
import sys
from contextlib import ExitStack

import numpy as np

for _p in ("/opt/trn_rl_repo",):
    if os.path.isdir(_p) and _p not in sys.path:
        sys.path.append(_p)

import concourse.bass as bass
import concourse.tile as tile
from concourse import bacc, mybir
from concourse.bass_utils import run_bass_kernel_spmd

B, N, D, H, HD = 2, 4096, 256, 8, 32
NEG_SLOPE = 0.01
LN_EPS = 1e-5
NCORES = 8
RPC = B * N // NCORES  # 1024 rows per core
RT = RPC // 128        # 8 row tiles per core
KT = D // 128          # 2 contraction tiles
F32 = mybir.dt.float32
AF = mybir.ActivationFunctionType
ALU = mybir.AluOpType
AX = mybir.AxisListType

_CACHE: dict = {}
last_results: list = []  # BassKernelResults of the most recent kernel() call


def _trace_args(tag):
    d = os.environ.get("GT_TRACE_DIR")
    if not d:
        return {}
    td = os.path.join(d, tag)
    os.makedirs(td, exist_ok=True)
    return {"trace": True, "tmpdir": td}


def _bcast(ap_1d, parts, n):
    """DRAM [n] -> broadcast access pattern [parts, n] (0-stride partitions)."""
    return bass.AP(tensor=ap_1d.tensor, offset=ap_1d.offset, ap=[[0, parts], [1, n]])


def _build_stage1():
    nc = bacc.Bacc("TRN2", target_bir_lowering=False, debug=False, num_devices=NCORES)
    hT = nc.dram_tensor("hT", [D, RPC], F32, kind="ExternalInput").ap()
    # wlr[k] = [W_l[k-tile] | W_r[k-tile]] : fused moving operand [128, 512]
    wlr = nc.dram_tensor("wlr", [D, 2 * D], F32, kind="ExternalInput").ap()
    # aflat = [tile(a_l,8)/H | tile(a_r,8)] : [512]
    aflat = nc.dram_tensor("aflat", [2 * D], F32, kind="ExternalInput").ap()
    slb = nc.dram_tensor("slb", [RPC], F32, kind="ExternalOutput").ap()
    srb = nc.dram_tensor("srb", [RPC], F32, kind="ExternalOutput").ap()
    sloc = nc.dram_tensor("sloc", [H], F32, kind="ExternalOutput").ap()
    csum = nc.dram_tensor("csum", [H, D], F32, kind="ExternalOutput").ap()

    with tile.TileContext(nc) as tc, ExitStack() as ctx:
        const = ctx.enter_context(tc.tile_pool(name="const", bufs=1))
        work = ctx.enter_context(tc.tile_pool(name="work", bufs=3))
        frp = ctx.enter_context(tc.tile_pool(name="frp", bufs=1))
        ps = ctx.enter_context(tc.tile_pool(name="ps", bufs=3, space="PSUM"))
        ps1 = ctx.enter_context(tc.tile_pool(name="ps1", bufs=1, space="PSUM"))

        sb_hT = []
        sb_wlr = []
        for k in range(KT):
            t = const.tile([128, RPC], F32, tag=f"hT{k}")
            nc.sync.dma_start(t[:], hT[k * 128:(k + 1) * 128, :])
            sb_hT.append(t)
            t = const.tile([128, 2 * D], F32, tag=f"wlr{k}")
            nc.sync.dma_start(t[:], wlr[k * 128:(k + 1) * 128, :])
            sb_wlr.append(t)
        sb_ab = const.tile([128, 2 * D], F32, tag="ab")
        nc.sync.dma_start(sb_ab[:], _bcast(aflat, 128, 2 * D))
        sb_ones = const.tile([128, 1], F32, tag="ones")
        nc.vector.memset(sb_ones[:], 1.0)

        sb_slc = const.tile([128, RT], F32, tag="slc")
        sb_src = const.tile([128, RT], F32, tag="src")
        pc = ps1.tile([H, D], F32, tag="csum")   # exp-weighted fr sum
        psm = ps1.tile([1, H], F32, tag="sloc")  # exp sums

        for jt in range(RT):
            # [fl | fr] rows for this row tile: psum [128, 512]
            pt = ps.tile([128, 2 * D], F32, tag="mm")
            for k in range(KT):
                nc.tensor.matmul(
                    pt[:],
                    sb_hT[k][:, jt * 128:(jt + 1) * 128],
                    sb_wlr[k][:],
                    start=(k == 0),
                    stop=(k == KT - 1),
                )
            # leaky(x) = 0.01*x + relu(0.99*x)
            rl = work.tile([128, 2 * D], F32, tag="rl")
            nc.scalar.activation(rl[:], pt[:], AF.Relu, scale=1.0 - NEG_SLOPE)
            lk = work.tile([128, 2 * D], F32, tag="lk")
            nc.vector.scalar_tensor_tensor(
                out=lk[:], in0=pt[:], scalar=NEG_SLOPE, in1=rl[:],
                op0=ALU.mult, op1=ALU.add,
            )
            # raw fr rows for the context contraction
            fr_raw = frp.tile([128, D], F32, tag=f"fr{jt}")
            nc.vector.tensor_copy(fr_raw[:], pt[:, D:2 * D])
            # score dots: t3 = leaky * [a_l/H | a_r]
            t3 = work.tile([128, 2 * D], F32, tag="t3")
            nc.vector.tensor_mul(t3[:], lk[:], sb_ab[:])
            nc.vector.reduce_sum(out=sb_slc[:, jt:jt + 1], in_=t3[:, 0:D], axis=AX.X)
            srh = work.tile([128, H], F32, tag="srh")
            nc.vector.reduce_sum(
                out=srh[:], in_=t3[:, D:2 * D].rearrange("p (h w) -> p h w", h=H),
                axis=AX.X,
            )
            nc.vector.reduce_sum(out=sb_src[:, jt:jt + 1], in_=srh[:], axis=AX.X)
            # e = exp(sr) (scores are O(1): no max shift; combined on host)
            er = work.tile([128, H], F32, tag="er")
            nc.scalar.activation(er[:], srh[:], AF.Exp)
            nc.tensor.matmul(psm[:], sb_ones[:], er[:],
                             start=(jt == 0), stop=(jt == RT - 1))
            nc.tensor.matmul(pc[:], er[:], fr_raw[:],
                             start=(jt == 0), stop=(jt == RT - 1))

        # head-mean for the attn column scores: sr_bar = sum_h srh / H
        sb_srm = const.tile([128, RT], F32, tag="srm")
        nc.scalar.mul(sb_srm[:], sb_src[:], 1.0 / H)

        sb_sloc = work.tile([1, H], F32, tag="slocv")
        nc.vector.tensor_copy(sb_sloc[:], psm[:])
        nc.sync.dma_start(sloc[None, :], sb_sloc[:])
        sb_csum = work.tile([H, D], F32, tag="csumv")
        nc.vector.tensor_copy(sb_csum[:], pc[:])
        nc.sync.dma_start(csum[:], sb_csum[:])
        # rows r = jt*128 + p  ->  DRAM AP [p, jt]
        nc.sync.dma_start(slb.rearrange("(jt p) -> p jt", p=128), sb_slc[:])
        nc.sync.dma_start(srb.rearrange("(jt p) -> p jt", p=128), sb_srm[:])

    nc.compile()
    return nc


def _build_stage2():
    nc = bacc.Bacc("TRN2", target_bir_lowering=False, debug=False, num_devices=NCORES)
    hrows = nc.dram_tensor("hrows", [RPC, D], F32, kind="ExternalInput").ap()
    slcol = nc.dram_tensor("slcol", [128, RT], F32, kind="ExternalInput").ap()
    srbv = nc.dram_tensor("srbv", [N], F32, kind="ExternalInput").ap()
    fhv = nc.dram_tensor("fhv", [D], F32, kind="ExternalInput").ap()
    lng = nc.dram_tensor("lng", [D], F32, kind="ExternalInput").ap()
    lnb = nc.dram_tensor("lnb", [D], F32, kind="ExternalInput").ap()
    attn = nc.dram_tensor("attn", [RPC, N], F32, kind="ExternalOutput").ap()
    hout = nc.dram_tensor("hout", [RPC, D], F32, kind="ExternalOutput").ap()

    NQ = 4          # srb broadcast chunks
    CH = N // NQ    # 1024

    with tile.TileContext(nc) as tc, ExitStack() as ctx:
        const = ctx.enter_context(tc.tile_pool(name="const", bufs=1))
        apool = ctx.enter_context(tc.tile_pool(name="apool", bufs=3))
        hpool = ctx.enter_context(tc.tile_pool(name="hpool", bufs=3))
        spool = ctx.enter_context(tc.tile_pool(name="spool", bufs=4))

        sb_sl = const.tile([128, RT], F32, tag="sl")
        nc.gpsimd.dma_start(sb_sl[:], slcol[:])
        sb_sr = const.tile([128, N], F32, tag="sr")
        for q in range(NQ):
            nc.sync.dma_start(
                sb_sr[:, q * CH:(q + 1) * CH],
                _bcast(srbv[q * CH:(q + 1) * CH], 128, CH),
            )
        sb_fh = const.tile([128, D], F32, tag="fh")
        nc.gpsimd.dma_start(sb_fh[:], _bcast(fhv, 128, D))
        sb_g = const.tile([128, D], F32, tag="g")
        nc.gpsimd.dma_start(sb_g[:], _bcast(lng, 128, D))
        sb_b = const.tile([128, D], F32, tag="b")
        nc.gpsimd.dma_start(sb_b[:], _bcast(lnb, 128, D))
        sb_eps = const.tile([128, 1], F32, tag="eps")
        nc.vector.memset(sb_eps[:], LN_EPS)

        for t in range(RT):
            rs_ = slice(t * 128, (t + 1) * 128)
            at = apool.tile([128, N], F32, tag="at")
            # outer sum, halves on different engines; DMA each half when ready,
            # alternating issue engines to spread DGE queues
            nc.vector.tensor_scalar_add(at[:, 0:N // 2], sb_sr[:, 0:N // 2], sb_sl[:, t:t + 1])
            nc.sync.dma_start(attn[rs_, 0:N // 2], at[:, 0:N // 2])
            nc.scalar.activation(at[:, N // 2:N], sb_sr[:, N // 2:N], AF.Identity, bias=sb_sl[:, t:t + 1])
            nc.scalar.dma_start(attn[rs_, N // 2:N], at[:, N // 2:N])

            ht = hpool.tile([128, D], F32, tag="ht")
            nc.gpsimd.dma_start(ht[:], hrows[rs_, :])
            xt = hpool.tile([128, D], F32, tag="xt")
            nc.vector.tensor_add(xt[:], ht[:], sb_fh[:])
            st = spool.tile([128, 6], F32, tag="st")
            nc.vector.bn_stats(out=st[:], in_=xt[:])
            mv = spool.tile([128, 2], F32, tag="mv")
            nc.vector.bn_aggr(out=mv[:], in_=st[:])
            rstd = spool.tile([128, 1], F32, tag="rstd")
            nc.scalar.activation(rstd[:], mv[:, 1:2], AF.Sqrt, bias=sb_eps[:])
            nc.vector.reciprocal(rstd[:], rstd[:])
            yt = hpool.tile([128, D], F32, tag="yt")
            nc.vector.tensor_scalar(
                out=yt[:], in0=xt[:],
                scalar1=mv[:, 0:1], scalar2=rstd[:],
                op0=ALU.subtract, op1=ALU.mult,
            )
            nc.vector.tensor_mul(yt[:], yt[:], sb_g[:])
            nc.vector.tensor_add(yt[:], yt[:], sb_b[:])
            nc.gpsimd.dma_start(hout[rs_, :], yt[:])

    nc.compile()
    return nc


def _get_programs():
    if "s1" not in _CACHE:
        _CACHE["s1"] = _build_stage1()
        _CACHE["s2"] = _build_stage2()
    return _CACHE["s1"], _CACHE["s2"]


def kernel(h, W_l, W_r, a_l, a_r, W_final, ln_g, ln_b):
    global last_results
    h = np.ascontiguousarray(np.asarray(h, np.float32))
    W_l = np.asarray(W_l, np.float32)
    W_r = np.asarray(W_r, np.float32)
    a_l = np.asarray(a_l, np.float32)
    a_r = np.asarray(a_r, np.float32)
    W_final = np.asarray(W_final, np.float32)
    ln_g = np.asarray(ln_g, np.float32)
    ln_b = np.asarray(ln_b, np.float32)

    nc1, nc2 = _get_programs()
    hf = h.reshape(B * N, D)

    wlr = np.concatenate([W_l, W_r], axis=1)
    aflat = np.concatenate([np.tile(a_l, H) / H, np.tile(a_r, H)]).astype(np.float32)

    in1 = []
    for c in range(NCORES):
        rows = hf[c * RPC:(c + 1) * RPC]
        in1.append({"hT": np.ascontiguousarray(rows.T), "wlr": wlr, "aflat": aflat})
    res1 = run_bass_kernel_spmd(nc1, in1, list(range(NCORES)), **_trace_args("s1"))
    r1 = res1.results

    slb = np.concatenate([r1[c]["slb"] for c in range(NCORES)])
    srbf = np.concatenate([r1[c]["srb"] for c in range(NCORES)])
    sloc = np.stack([r1[c]["sloc"] for c in range(NCORES)])   # [8, H]
    csum = np.stack([r1[c]["csum"] for c in range(NCORES)])   # [8, H, D]

    # global softmax combine + (degenerate, one row per batch) W_final matmul
    fh = np.zeros((B, D), np.float32)
    for b in range(B):
        cs = slice(b * (NCORES // B), (b + 1) * (NCORES // B))
        S = sloc[cs].sum(axis=0)                  # [H]
        Cs = csum[cs].sum(axis=0)                 # [H, D]
        cvec = np.stack([Cs[hh, HD * hh:HD * (hh + 1)] / S[hh] for hh in range(H)])
        fh[b] = cvec.reshape(D).astype(np.float32) @ W_final

    sr_bar = srbf.reshape(B, N)

    in2 = []
    for c in range(NCORES):
        b = c // (NCORES // B)
        sl_own = slb[c * RPC:(c + 1) * RPC]
        in2.append({
            "hrows": hf[c * RPC:(c + 1) * RPC],
            "slcol": np.ascontiguousarray(sl_own.reshape(RT, 128).T),
            "srbv": sr_bar[b],
            "fhv": fh[b],
            "lng": ln_g,
            "lnb": ln_b,
        })
    res2 = run_bass_kernel_spmd(nc2, in2, list(range(NCORES)), **_trace_args("s2"))
    r2 = res2.results
    last_results = [res1, res2]

    attn = np.concatenate([r2[c]["attn"] for c in range(NCORES)]).reshape(B, N, N)
    hout = np.concatenate([r2[c]["hout"] for c in range(NCORES)]).reshape(B, N, D)
    return hout, attn


# revision 16
# speedup vs baseline: 1.2345x; 1.2200x over previous
"""Trainium2 Bass kernel for the GAT-style transformer layer (nn_GTLayer).

Math used (exact restructuring of the reference):
  score[b,h,i,j] = sl[b,h,i] + sr[b,h,j]  with
      sl = leaky(h@W_l) . a_l (per head),  sr = leaky(h@W_r) . a_r
  - softmax_j(score) = softmax_j(sr[b,h,:]) : independent of i  (shift
    invariance), so context[b,h,i,:] = c[b,h,:] = sum_j w_j fr[b,h,j,:]
    for every i, and fh = (concat_h c) @ W_final is one row per batch.
  - attn = mean_h score = sl_bar[b,i] + sr_bar[b,j]  (rank-1 outer sum).

So the kernel is memory-bound on writing attn [B,N,N] (134 MB fp32).

Two SPMD launches over 8 cores (rows of (B*N) sharded, 1024 rows/core,
each core's rows inside one batch):
  stage 1: per-core fl/fr projections in [row, dout] layout (one fused
           [W_l|W_r] moving operand per k-tile), leaky + score dots on
           Vector, softmax partials (sum-exp and exp-weighted fr sum —
           scores are O(1) so no max shift is needed) via PE contractions.
  host:    combine ~18 KB of per-core softmax partials, tiny c @ W_final.
  stage 2: stream attn rows = sl_bar[i] + sr_bar[j] (outer sum, halves
           split across Vector+Scalar engines) and h_out = LN(h+fh) rows.
"""

import os
import sys
from contextlib import ExitStack

import numpy as np

for _p in ("/opt/trn_rl_repo",):
    if os.path.isdir(_p) and _p not in sys.path:
        sys.path.append(_p)

import concourse.bass as bass
import concourse.tile as tile
from concourse import bacc, mybir
from concourse.bass_utils import run_bass_kernel_spmd

B, N, D, H, HD = 2, 4096, 256, 8, 32
NEG_SLOPE = 0.01
LN_EPS = 1e-5
NCORES = 8
RPC = B * N // NCORES  # 1024 rows per core
RT = RPC // 128        # 8 row tiles per core
KT = D // 128          # 2 contraction tiles
F32 = mybir.dt.float32
AF = mybir.ActivationFunctionType
ALU = mybir.AluOpType
AX = mybir.AxisListType

_CACHE: dict = {}
last_results: list = []  # BassKernelResults of the most recent kernel() call


def _trace_args(tag):
    d = os.environ.get("GT_TRACE_DIR")
    if not d:
        return {}
    td = os.path.join(d, tag)
    os.makedirs(td, exist_ok=True)
    return {"trace": True, "tmpdir": td}


def _bcast(ap_1d, parts, n):
    """DRAM [n] -> broadcast access pattern [parts, n] (0-stride partitions)."""
    return bass.AP(tensor=ap_1d.tensor, offset=ap_1d.offset, ap=[[0, parts], [1, n]])


def _build_stage1():
    nc = bacc.Bacc("TRN2", target_bir_lowering=False, debug=False, num_devices=NCORES)
    hT = nc.dram_tensor("hT", [D, RPC], F32, kind="ExternalInput").ap()
    # wlr[k] = [W_l[k-tile] | W_r[k-tile]] : fused moving operand [128, 512]
    wlr = nc.dram_tensor("wlr", [D, 2 * D], F32, kind="ExternalInput").ap()
    # aflat = [tile(a_l,8)/H | tile(a_r,8)] : [512]
    aflat = nc.dram_tensor("aflat", [2 * D], F32, kind="ExternalInput").ap()
    # [p, jt] layout: row r = jt*128 + p lives at [r % 128, r // 128]
    slb = nc.dram_tensor("slb", [128, RT], F32, kind="ExternalOutput").ap()
    srb = nc.dram_tensor("srb", [128, RT], F32, kind="ExternalOutput").ap()
    sloc = nc.dram_tensor("sloc", [H], F32, kind="ExternalOutput").ap()
    csum = nc.dram_tensor("csum", [H, D], F32, kind="ExternalOutput").ap()

    with tile.TileContext(nc) as tc, ExitStack() as ctx:
        const = ctx.enter_context(tc.tile_pool(name="const", bufs=1))
        work = ctx.enter_context(tc.tile_pool(name="work", bufs=3))
        frp = ctx.enter_context(tc.tile_pool(name="frp", bufs=1))
        ps = ctx.enter_context(tc.tile_pool(name="ps", bufs=5, space="PSUM"))
        ps1 = ctx.enter_context(tc.tile_pool(name="ps1", bufs=1, space="PSUM"))

        sb_hT = []
        sb_wlr = []
        for k in range(KT):
            t = const.tile([128, RPC], F32, tag=f"hT{k}")
            nc.sync.dma_start(t[:], hT[k * 128:(k + 1) * 128, :])
            sb_hT.append(t)
            t = const.tile([128, 2 * D], F32, tag=f"wlr{k}")
            nc.sync.dma_start(t[:], wlr[k * 128:(k + 1) * 128, :])
            sb_wlr.append(t)
        sb_ab = const.tile([128, 2 * D], F32, tag="ab")
        nc.sync.dma_start(sb_ab[:], _bcast(aflat, 128, 2 * D))
        sb_ones = const.tile([128, 1], F32, tag="ones")
        nc.vector.memset(sb_ones[:], 1.0)

        sb_slc = const.tile([128, RT], F32, tag="slc")
        sb_src = const.tile([128, RT], F32, tag="src")
        pc = ps1.tile([H, D], F32, tag="csum")   # exp-weighted fr sum
        psm = ps1.tile([1, H], F32, tag="sloc")  # exp sums

        for jt in range(RT):
            # [fl | fr] rows for this row tile: psum [128, 512]
            pt = ps.tile([128, 2 * D], F32, tag="mm")
            for k in range(KT):
                nc.tensor.matmul(
                    pt[:],
                    sb_hT[k][:, jt * 128:(jt + 1) * 128],
                    sb_wlr[k][:],
                    start=(k == 0),
                    stop=(k == KT - 1),
                )
            # leaky(x) = 0.01*x + relu(0.99*x)
            rl = work.tile([128, 2 * D], F32, tag="rl")
            nc.scalar.activation(rl[:], pt[:], AF.Relu, scale=1.0 - NEG_SLOPE)
            lk = work.tile([128, 2 * D], F32, tag="lk")
            nc.vector.scalar_tensor_tensor(
                out=lk[:], in0=pt[:], scalar=NEG_SLOPE, in1=rl[:],
                op0=ALU.mult, op1=ALU.add,
            )
            # raw fr rows for the context contraction (ScalarE has slack)
            fr_raw = frp.tile([128, D], F32, tag=f"fr{jt}")
            nc.scalar.copy(fr_raw[:], pt[:, D:2 * D])
            # score dots: sl on Vector (fused mul+reduce), sr on GpSimd
            t3l = work.tile([128, D], F32, tag="t3l")
            nc.vector.tensor_mul(t3l[:], lk[:, 0:D], sb_ab[:, 0:D])
            nc.vector.reduce_sum(out=sb_slc[:, jt:jt + 1], in_=t3l[:], axis=AX.X)
            t3r = work.tile([128, D], F32, tag="t3r")
            nc.vector.tensor_mul(t3r[:], lk[:, D:2 * D], sb_ab[:, D:2 * D])
            srh = work.tile([128, H], F32, tag="srh")
            nc.vector.reduce_sum(
                out=srh[:], in_=t3r[:].rearrange("p (h w) -> p h w", h=H),
                axis=AX.X,
            )
            nc.vector.reduce_sum(out=sb_src[:, jt:jt + 1], in_=srh[:], axis=AX.X)
            # e = exp(sr) (scores are O(1): no max shift; combined on host)
            er = work.tile([128, H], F32, tag="er")
            nc.scalar.activation(er[:], srh[:], AF.Exp)
            nc.tensor.matmul(psm[:], sb_ones[:], er[:],
                             start=(jt == 0), stop=(jt == RT - 1))
            nc.tensor.matmul(pc[:], er[:], fr_raw[:],
                             start=(jt == 0), stop=(jt == RT - 1))

        # head-mean for the attn column scores: sr_bar = sum_h srh / H
        sb_srm = const.tile([128, RT], F32, tag="srm")
        nc.scalar.mul(sb_srm[:], sb_src[:], 1.0 / H)

        sb_sloc = work.tile([1, H], F32, tag="slocv")
        nc.vector.tensor_copy(sb_sloc[:], psm[:])
        nc.sync.dma_start(sloc[None, :], sb_sloc[:])
        sb_csum = work.tile([H, D], F32, tag="csumv")
        nc.vector.tensor_copy(sb_csum[:], pc[:])
        nc.sync.dma_start(csum[:], sb_csum[:])
        nc.sync.dma_start(slb[:], sb_slc[:])
        nc.sync.dma_start(srb[:], sb_srm[:])

    nc.compile()
    return nc


def _build_stage2():
    nc = bacc.Bacc("TRN2", target_bir_lowering=False, debug=False, num_devices=NCORES)
    hrows = nc.dram_tensor("hrows", [RPC, D], F32, kind="ExternalInput").ap()
    slcol = nc.dram_tensor("slcol", [128, RT], F32, kind="ExternalInput").ap()
    srbv = nc.dram_tensor("srbv", [N], F32, kind="ExternalInput").ap()
    fhv = nc.dram_tensor("fhv", [D], F32, kind="ExternalInput").ap()
    lng = nc.dram_tensor("lng", [D], F32, kind="ExternalInput").ap()
    lnb = nc.dram_tensor("lnb", [D], F32, kind="ExternalInput").ap()
    attn = nc.dram_tensor("attn", [RPC, N], F32, kind="ExternalOutput").ap()
    hout = nc.dram_tensor("hout", [RPC, D], F32, kind="ExternalOutput").ap()

    NQ = 4          # srb broadcast chunks
    CH = N // NQ    # 1024

    with tile.TileContext(nc) as tc, ExitStack() as ctx:
        const = ctx.enter_context(tc.tile_pool(name="const", bufs=1))
        apool = ctx.enter_context(tc.tile_pool(name="apool", bufs=3))
        hpool = ctx.enter_context(tc.tile_pool(name="hpool", bufs=3))
        spool = ctx.enter_context(tc.tile_pool(name="spool", bufs=4))

        sb_sl = const.tile([128, RT], F32, tag="sl")
        nc.gpsimd.dma_start(sb_sl[:], slcol[:])
        sb_sr = const.tile([128, N], F32, tag="sr")
        for q in range(NQ):
            nc.sync.dma_start(
                sb_sr[:, q * CH:(q + 1) * CH],
                _bcast(srbv[q * CH:(q + 1) * CH], 128, CH),
            )
        sb_fh = const.tile([128, D], F32, tag="fh")
        nc.gpsimd.dma_start(sb_fh[:], _bcast(fhv, 128, D))
        sb_g = const.tile([128, D], F32, tag="g")
        nc.gpsimd.dma_start(sb_g[:], _bcast(lng, 128, D))
        sb_b = const.tile([128, D], F32, tag="b")
        nc.gpsimd.dma_start(sb_b[:], _bcast(lnb, 128, D))
        sb_eps = const.tile([128, 1], F32, tag="eps")
        nc.vector.memset(sb_eps[:], LN_EPS)

        for t in range(RT):
            rs_ = slice(t * 128, (t + 1) * 128)
            at = apool.tile([128, N], F32, tag="at")
            # outer sum, halves on different engines; DMA each half when ready,
            # alternating issue engines to spread DGE queues
            nc.vector.tensor_scalar_add(at[:, 0:N // 2], sb_sr[:, 0:N // 2], sb_sl[:, t:t + 1])
            nc.sync.dma_start(attn[rs_, 0:N // 2], at[:, 0:N // 2])
            nc.scalar.activation(at[:, N // 2:N], sb_sr[:, N // 2:N], AF.Identity, bias=sb_sl[:, t:t + 1])
            nc.scalar.dma_start(attn[rs_, N // 2:N], at[:, N // 2:N])

            ht = hpool.tile([128, D], F32, tag="ht")
            nc.gpsimd.dma_start(ht[:], hrows[rs_, :])
            xt = hpool.tile([128, D], F32, tag="xt")
            nc.vector.tensor_add(xt[:], ht[:], sb_fh[:])
            st = spool.tile([128, 6], F32, tag="st")
            nc.vector.bn_stats(out=st[:], in_=xt[:])
            mv = spool.tile([128, 2], F32, tag="mv")
            nc.vector.bn_aggr(out=mv[:], in_=st[:])
            rstd = spool.tile([128, 1], F32, tag="rstd")
            nc.scalar.activation(rstd[:], mv[:, 1:2], AF.Sqrt, bias=sb_eps[:])
            nc.vector.reciprocal(rstd[:], rstd[:])
            yt = hpool.tile([128, D], F32, tag="yt")
            nc.vector.tensor_scalar(
                out=yt[:], in0=xt[:],
                scalar1=mv[:, 0:1], scalar2=rstd[:],
                op0=ALU.subtract, op1=ALU.mult,
            )
            nc.vector.tensor_mul(yt[:], yt[:], sb_g[:])
            nc.vector.tensor_add(yt[:], yt[:], sb_b[:])
            nc.gpsimd.dma_start(hout[rs_, :], yt[:])

    nc.compile()
    return nc


def _get_programs():
    if "s1" not in _CACHE:
        _CACHE["s1"] = _build_stage1()
        _CACHE["s2"] = _build_stage2()
    return _CACHE["s1"], _CACHE["s2"]


def kernel(h, W_l, W_r, a_l, a_r, W_final, ln_g, ln_b):
    global last_results
    h = np.ascontiguousarray(np.asarray(h, np.float32))
    W_l = np.asarray(W_l, np.float32)
    W_r = np.asarray(W_r, np.float32)
    a_l = np.asarray(a_l, np.float32)
    a_r = np.asarray(a_r, np.float32)
    W_final = np.asarray(W_final, np.float32)
    ln_g = np.asarray(ln_g, np.float32)
    ln_b = np.asarray(ln_b, np.float32)

    nc1, nc2 = _get_programs()
    hf = h.reshape(B * N, D)

    wlr = np.concatenate([W_l, W_r], axis=1)
    aflat = np.concatenate([np.tile(a_l, H) / H, np.tile(a_r, H)]).astype(np.float32)

    in1 = []
    for c in range(NCORES):
        rows = hf[c * RPC:(c + 1) * RPC]
        in1.append({"hT": np.ascontiguousarray(rows.T), "wlr": wlr, "aflat": aflat})
    res1 = run_bass_kernel_spmd(nc1, in1, list(range(NCORES)), **_trace_args("s1"))
    r1 = res1.results

    # slb/srb are [128, RT] per core with row r at [r % 128, r // 128]
    srbf = np.concatenate([r1[c]["srb"].T.reshape(-1) for c in range(NCORES)])
    sloc = np.stack([r1[c]["sloc"] for c in range(NCORES)])   # [8, H]
    csum = np.stack([r1[c]["csum"] for c in range(NCORES)])   # [8, H, D]

    # global softmax combine + (degenerate, one row per batch) W_final matmul
    fh = np.zeros((B, D), np.float32)
    for b in range(B):
        cs = slice(b * (NCORES // B), (b + 1) * (NCORES // B))
        S = sloc[cs].sum(axis=0)                  # [H]
        Cs = csum[cs].sum(axis=0)                 # [H, D]
        cvec = np.stack([Cs[hh, HD * hh:HD * (hh + 1)] / S[hh] for hh in range(H)])
        fh[b] = cvec.reshape(D).astype(np.float32) @ W_final

    sr_bar = srbf.reshape(B, N)

    in2 = []
    for c in range(NCORES):
        b = c // (NCORES // B)
        in2.append({
            "hrows": hf[c * RPC:(c + 1) * RPC],
            "slcol": r1[c]["slb"],   # already [128, RT] = [p, jt]
            "srbv": sr_bar[b],
            "fhv": fh[b],
            "lng": ln_g,
            "lnb": ln_b,
        })
    res2 = run_bass_kernel_spmd(nc2, in2, list(range(NCORES)), **_trace_args("s2"))
    r2 = res2.results
    last_results = [res1, res2]

    attn = np.concatenate([r2[c]["attn"] for c in range(NCORES)]).reshape(B, N, N)
    hout = np.concatenate([r2[c]["hout"] for c in range(NCORES)]).reshape(B, N, D)
    return hout, attn


# revision 19
# speedup vs baseline: 1.3316x; 1.0787x over previous
"""Trainium2 Bass kernel for the GAT-style transformer layer (nn_GTLayer).

Math used (exact restructuring of the reference):
  score[b,h,i,j] = sl[b,h,i] + sr[b,h,j]  with
      sl = leaky(h@W_l) . a_l (per head),  sr = leaky(h@W_r) . a_r
  - softmax_j(score) = softmax_j(sr[b,h,:]) : independent of i  (shift
    invariance), so context[b,h,i,:] = c[b,h,:] = sum_j w_j fr[b,h,j,:]
    for every i, and fh = (concat_h c) @ W_final is one row per batch.
  - attn = mean_h score = sl_bar[b,i] + sr_bar[b,j]  (rank-1 outer sum).

So the kernel is memory-bound on writing attn [B,N,N] (134 MB fp32).

Two SPMD launches over 8 cores (rows of (B*N) sharded, 1024 rows/core,
each core's rows inside one batch):
  stage 1: per-core fl/fr projections in [row, dout] layout (one fused
           [W_l|W_r] moving operand per k-tile), leaky + score dots on
           Vector, softmax partials (sum-exp and exp-weighted fr sum —
           scores are O(1) so no max shift is needed) via PE contractions.
  host:    combine ~18 KB of per-core softmax partials, tiny c @ W_final.
  stage 2: stream attn rows = sl_bar[i] + sr_bar[j] (outer sum, halves
           split across Vector+Scalar engines) and h_out = LN(h+fh) rows.
"""

import os
import sys
from contextlib import ExitStack

import numpy as np

for _p in ("/opt/trn_rl_repo",):
    if os.path.isdir(_p) and _p not in sys.path:
        sys.path.append(_p)

import concourse.bass as bass
import concourse.tile as tile
from concourse import bacc, mybir
from concourse.bass_utils import run_bass_kernel_spmd

B, N, D, H, HD = 2, 4096, 256, 8, 32
NEG_SLOPE = 0.01
LN_EPS = 1e-5
NCORES = 8
RPC = B * N // NCORES  # 1024 rows per core
RT = RPC // 128        # 8 row tiles per core
KT = D // 128          # 2 contraction tiles
F32 = mybir.dt.float32
AF = mybir.ActivationFunctionType
ALU = mybir.AluOpType
AX = mybir.AxisListType

_CACHE: dict = {}
last_results: list = []  # BassKernelResults of the most recent kernel() call


def _trace_args(tag):
    d = os.environ.get("GT_TRACE_DIR")
    if not d:
        return {}
    td = os.path.join(d, tag)
    os.makedirs(td, exist_ok=True)
    return {"trace": True, "tmpdir": td}


def _bcast(ap_1d, parts, n):
    """DRAM [n] -> broadcast access pattern [parts, n] (0-stride partitions)."""
    return bass.AP(tensor=ap_1d.tensor, offset=ap_1d.offset, ap=[[0, parts], [1, n]])


def _build_stage1():
    nc = bacc.Bacc("TRN2", target_bir_lowering=False, debug=False, num_devices=NCORES)
    hT = nc.dram_tensor("hT", [D, RPC], F32, kind="ExternalInput").ap()
    # wlr[k] = [W_l[k-tile] | W_r[k-tile]] : fused moving operand [128, 512]
    wlr = nc.dram_tensor("wlr", [D, 2 * D], F32, kind="ExternalInput").ap()
    # aflat = [tile(a_l,8)/H | tile(a_r,8)] : [512]
    aflat = nc.dram_tensor("aflat", [2 * D], F32, kind="ExternalInput").ap()
    # [p, jt] layout: row r = jt*128 + p lives at [r % 128, r // 128]
    slb = nc.dram_tensor("slb", [128, RT], F32, kind="ExternalOutput").ap()
    srb = nc.dram_tensor("srb", [128, RT], F32, kind="ExternalOutput").ap()
    sloc = nc.dram_tensor("sloc", [H], F32, kind="ExternalOutput").ap()
    csum = nc.dram_tensor("csum", [H, D], F32, kind="ExternalOutput").ap()

    with tile.TileContext(nc) as tc, ExitStack() as ctx:
        const = ctx.enter_context(tc.tile_pool(name="const", bufs=1))
        work = ctx.enter_context(tc.tile_pool(name="work", bufs=3))
        frp = ctx.enter_context(tc.tile_pool(name="frp", bufs=1))
        ps = ctx.enter_context(tc.tile_pool(name="ps", bufs=5, space="PSUM"))
        ps1 = ctx.enter_context(tc.tile_pool(name="ps1", bufs=1, space="PSUM"))

        sb_hT = []
        sb_wlr = []
        for k in range(KT):
            t = const.tile([128, RPC], F32, tag=f"hT{k}")
            nc.sync.dma_start(t[:], hT[k * 128:(k + 1) * 128, :])
            sb_hT.append(t)
            t = const.tile([128, 2 * D], F32, tag=f"wlr{k}")
            nc.sync.dma_start(t[:], wlr[k * 128:(k + 1) * 128, :])
            sb_wlr.append(t)
        sb_ab = const.tile([128, 2 * D], F32, tag="ab")
        nc.sync.dma_start(sb_ab[:], _bcast(aflat, 128, 2 * D))
        sb_ones = const.tile([128, 1], F32, tag="ones")
        nc.vector.memset(sb_ones[:], 1.0)

        sb_slc = const.tile([128, RT], F32, tag="slc")
        sb_src = const.tile([128, RT], F32, tag="src")
        pc = ps1.tile([H, D], F32, tag="csum")   # exp-weighted fr sum
        psm = ps1.tile([1, H], F32, tag="sloc")  # exp sums

        ers = []
        for jt in range(RT):
            # [fl | fr] rows for this row tile: psum [128, 512]
            pt = ps.tile([128, 2 * D], F32, tag="mm")
            for k in range(KT):
                nc.tensor.matmul(
                    pt[:],
                    sb_hT[k][:, jt * 128:(jt + 1) * 128],
                    sb_wlr[k][:],
                    start=(k == 0),
                    stop=(k == KT - 1),
                )
            # leaky(x) = 0.01*x + relu(0.99*x)
            rl = work.tile([128, 2 * D], F32, tag="rl")
            nc.scalar.activation(rl[:], pt[:], AF.Relu, scale=1.0 - NEG_SLOPE)
            lk = work.tile([128, 2 * D], F32, tag="lk")
            nc.vector.scalar_tensor_tensor(
                out=lk[:], in0=pt[:], scalar=NEG_SLOPE, in1=rl[:],
                op0=ALU.mult, op1=ALU.add,
            )
            # raw fr rows for the context contraction (ScalarE has slack)
            fr_raw = frp.tile([128, D], F32, tag=f"fr{jt}")
            nc.scalar.copy(fr_raw[:], pt[:, D:2 * D])
            # score dots: sl on Vector (fused mul+reduce), sr on GpSimd
            t3l = work.tile([128, D], F32, tag="t3l")
            nc.vector.tensor_mul(t3l[:], lk[:, 0:D], sb_ab[:, 0:D])
            nc.vector.reduce_sum(out=sb_slc[:, jt:jt + 1], in_=t3l[:], axis=AX.X)
            t3r = work.tile([128, D], F32, tag="t3r")
            nc.vector.tensor_mul(t3r[:], lk[:, D:2 * D], sb_ab[:, D:2 * D])
            srh = work.tile([128, H], F32, tag="srh")
            nc.vector.reduce_sum(
                out=srh[:], in_=t3r[:].rearrange("p (h w) -> p h w", h=H),
                axis=AX.X,
            )
            nc.vector.reduce_sum(out=sb_src[:, jt:jt + 1], in_=srh[:], axis=AX.X)
            # e = exp(sr) (scores are O(1): no max shift; combined on host)
            er = frp.tile([128, H], F32, tag=f"er{jt}")
            nc.scalar.activation(er[:], srh[:], AF.Exp)
            ers.append((er, fr_raw))

        # trailing contractions, emitted after all projections so the PE
        # stream stays dense (HAM warm) instead of stalling on each jt chain
        for jt, (er, fr_raw) in enumerate(ers):
            nc.tensor.matmul(psm[:], sb_ones[:], er[:],
                             start=(jt == 0), stop=(jt == RT - 1))
            nc.tensor.matmul(pc[:], er[:], fr_raw[:],
                             start=(jt == 0), stop=(jt == RT - 1))

        # head-mean for the attn column scores: sr_bar = sum_h srh / H
        sb_srm = const.tile([128, RT], F32, tag="srm")
        nc.scalar.mul(sb_srm[:], sb_src[:], 1.0 / H)

        sb_sloc = work.tile([1, H], F32, tag="slocv")
        nc.vector.tensor_copy(sb_sloc[:], psm[:])
        nc.sync.dma_start(sloc[None, :], sb_sloc[:])
        sb_csum = work.tile([H, D], F32, tag="csumv")
        nc.vector.tensor_copy(sb_csum[:], pc[:])
        nc.sync.dma_start(csum[:], sb_csum[:])
        nc.sync.dma_start(slb[:], sb_slc[:])
        nc.sync.dma_start(srb[:], sb_srm[:])

    nc.compile()
    return nc


def _build_stage2():
    nc = bacc.Bacc("TRN2", target_bir_lowering=False, debug=False, num_devices=NCORES)
    hrows = nc.dram_tensor("hrows", [RPC, D], F32, kind="ExternalInput").ap()
    slcol = nc.dram_tensor("slcol", [128, RT], F32, kind="ExternalInput").ap()
    srbv = nc.dram_tensor("srbv", [N], F32, kind="ExternalInput").ap()
    fhv = nc.dram_tensor("fhv", [D], F32, kind="ExternalInput").ap()
    lng = nc.dram_tensor("lng", [D], F32, kind="ExternalInput").ap()
    lnb = nc.dram_tensor("lnb", [D], F32, kind="ExternalInput").ap()
    attn = nc.dram_tensor("attn", [RPC, N], F32, kind="ExternalOutput").ap()
    hout = nc.dram_tensor("hout", [RPC, D], F32, kind="ExternalOutput").ap()

    NQ = 4          # srb broadcast chunks
    CH = N // NQ    # 1024

    with tile.TileContext(nc) as tc, ExitStack() as ctx:
        const = ctx.enter_context(tc.tile_pool(name="const", bufs=1))
        apool = ctx.enter_context(tc.tile_pool(name="apool", bufs=3))
        hpool = ctx.enter_context(tc.tile_pool(name="hpool", bufs=3))
        spool = ctx.enter_context(tc.tile_pool(name="spool", bufs=4))

        sb_sl = const.tile([128, RT], F32, tag="sl")
        nc.sync.dma_start(sb_sl[:], slcol[:])
        sb_sr = const.tile([128, N], F32, tag="sr")
        for q in range(NQ):
            nc.sync.dma_start(
                sb_sr[:, q * CH:(q + 1) * CH],
                _bcast(srbv[q * CH:(q + 1) * CH], 128, CH),
            )
        sb_fh = const.tile([128, D], F32, tag="fh")
        nc.sync.dma_start(sb_fh[:], _bcast(fhv, 128, D))
        sb_g = const.tile([128, D], F32, tag="g")
        nc.sync.dma_start(sb_g[:], _bcast(lng, 128, D))
        sb_b = const.tile([128, D], F32, tag="b")
        nc.sync.dma_start(sb_b[:], _bcast(lnb, 128, D))
        sb_eps = const.tile([128, 1], F32, tag="eps")
        nc.vector.memset(sb_eps[:], LN_EPS)

        for t in range(RT):
            rs_ = slice(t * 128, (t + 1) * 128)
            at = apool.tile([128, N], F32, tag="at")
            # outer sum, halves on different engines; DMA each half when ready,
            # alternating issue engines to spread DGE queues
            nc.vector.tensor_scalar_add(at[:, 0:N // 2], sb_sr[:, 0:N // 2], sb_sl[:, t:t + 1])
            nc.sync.dma_start(attn[rs_, 0:N // 2], at[:, 0:N // 2])
            nc.scalar.activation(at[:, N // 2:N], sb_sr[:, N // 2:N], AF.Identity, bias=sb_sl[:, t:t + 1])
            nc.scalar.dma_start(attn[rs_, N // 2:N], at[:, N // 2:N])

            ht = hpool.tile([128, D], F32, tag="ht")
            nc.sync.dma_start(ht[:], hrows[rs_, :])
            xt = hpool.tile([128, D], F32, tag="xt")
            nc.vector.tensor_add(xt[:], ht[:], sb_fh[:])
            st = spool.tile([128, 6], F32, tag="st")
            nc.vector.bn_stats(out=st[:], in_=xt[:])
            mv = spool.tile([128, 2], F32, tag="mv")
            nc.vector.bn_aggr(out=mv[:], in_=st[:])
            rstd = spool.tile([128, 1], F32, tag="rstd")
            nc.scalar.activation(rstd[:], mv[:, 1:2], AF.Sqrt, bias=sb_eps[:])
            nc.vector.reciprocal(rstd[:], rstd[:])
            yt = hpool.tile([128, D], F32, tag="yt")
            nc.vector.tensor_scalar(
                out=yt[:], in0=xt[:],
                scalar1=mv[:, 0:1], scalar2=rstd[:],
                op0=ALU.subtract, op1=ALU.mult,
            )
            nc.vector.tensor_mul(yt[:], yt[:], sb_g[:])
            nc.vector.tensor_add(yt[:], yt[:], sb_b[:])
            nc.sync.dma_start(hout[rs_, :], yt[:])

    nc.compile()
    return nc


def _get_programs():
    if "s1" not in _CACHE:
        _CACHE["s1"] = _build_stage1()
        _CACHE["s2"] = _build_stage2()
    return _CACHE["s1"], _CACHE["s2"]


def kernel(h, W_l, W_r, a_l, a_r, W_final, ln_g, ln_b):
    global last_results
    h = np.ascontiguousarray(np.asarray(h, np.float32))
    W_l = np.asarray(W_l, np.float32)
    W_r = np.asarray(W_r, np.float32)
    a_l = np.asarray(a_l, np.float32)
    a_r = np.asarray(a_r, np.float32)
    W_final = np.asarray(W_final, np.float32)
    ln_g = np.asarray(ln_g, np.float32)
    ln_b = np.asarray(ln_b, np.float32)

    nc1, nc2 = _get_programs()
    hf = h.reshape(B * N, D)

    wlr = np.concatenate([W_l, W_r], axis=1)
    aflat = np.concatenate([np.tile(a_l, H) / H, np.tile(a_r, H)]).astype(np.float32)

    in1 = []
    for c in range(NCORES):
        rows = hf[c * RPC:(c + 1) * RPC]
        in1.append({"hT": np.ascontiguousarray(rows.T), "wlr": wlr, "aflat": aflat})
    res1 = run_bass_kernel_spmd(nc1, in1, list(range(NCORES)), **_trace_args("s1"))
    r1 = res1.results

    # slb/srb are [128, RT] per core with row r at [r % 128, r // 128]
    srbf = np.concatenate([r1[c]["srb"].T.reshape(-1) for c in range(NCORES)])
    sloc = np.stack([r1[c]["sloc"] for c in range(NCORES)])   # [8, H]
    csum = np.stack([r1[c]["csum"] for c in range(NCORES)])   # [8, H, D]

    # global softmax combine + (degenerate, one row per batch) W_final matmul
    fh = np.zeros((B, D), np.float32)
    for b in range(B):
        cs = slice(b * (NCORES // B), (b + 1) * (NCORES // B))
        S = sloc[cs].sum(axis=0)                  # [H]
        Cs = csum[cs].sum(axis=0)                 # [H, D]
        cvec = np.stack([Cs[hh, HD * hh:HD * (hh + 1)] / S[hh] for hh in range(H)])
        fh[b] = cvec.reshape(D).astype(np.float32) @ W_final

    sr_bar = srbf.reshape(B, N)

    in2 = []
    for c in range(NCORES):
        b = c // (NCORES // B)
        in2.append({
            "hrows": hf[c * RPC:(c + 1) * RPC],
            "slcol": r1[c]["slb"],   # already [128, RT] = [p, jt]
            "srbv": sr_bar[b],
            "fhv": fh[b],
            "lng": ln_g,
            "lnb": ln_b,
        })
    res2 = run_bass_kernel_spmd(nc2, in2, list(range(NCORES)), **_trace_args("s2"))
    r2 = res2.results
    last_results = [res1, res2]

    attn = np.concatenate([r2[c]["attn"] for c in range(NCORES)]).reshape(B, N, N)
    hout = np.concatenate([r2[c]["hout"] for c in range(NCORES)]).reshape(B, N, D)
    return hout, attn


# revision 20
# speedup vs baseline: 1.3575x; 1.0194x over previous
"""Trainium2 Bass kernel for the GAT-style transformer layer (nn_GTLayer).

Math used (exact restructuring of the reference):
  score[b,h,i,j] = sl[b,h,i] + sr[b,h,j]  with
      sl = leaky(h@W_l) . a_l (per head),  sr = leaky(h@W_r) . a_r
  - softmax_j(score) = softmax_j(sr[b,h,:]) : independent of i  (shift
    invariance), so context[b,h,i,:] = c[b,h,:] = sum_j w_j fr[b,h,j,:]
    for every i, and fh = (concat_h c) @ W_final is one row per batch.
  - attn = mean_h score = sl_bar[b,i] + sr_bar[b,j]  (rank-1 outer sum).

So the kernel is memory-bound on writing attn [B,N,N] (134 MB fp32).

Two SPMD launches over 8 cores (rows of (B*N) sharded, 1024 rows/core,
each core's rows inside one batch):
  stage 1: per-core fl/fr projections in [row, dout] layout (one fused
           [W_l|W_r] moving operand per k-tile), leaky + score dots on
           Vector, softmax partials (sum-exp and exp-weighted fr sum —
           scores are O(1) so no max shift is needed) via PE contractions.
  host:    combine ~18 KB of per-core softmax partials, tiny c @ W_final.
  stage 2: stream attn rows = sl_bar[i] + sr_bar[j] (outer sum, halves
           split across Vector+Scalar engines) and h_out = LN(h+fh) rows.
"""

import os
import sys
from contextlib import ExitStack

import numpy as np

for _p in ("/opt/trn_rl_repo",):
    if os.path.isdir(_p) and _p not in sys.path:
        sys.path.append(_p)

import concourse.bass as bass
import concourse.tile as tile
from concourse import bacc, mybir
from concourse.bass_utils import run_bass_kernel_spmd

B, N, D, H, HD = 2, 4096, 256, 8, 32
NEG_SLOPE = 0.01
LN_EPS = 1e-5
NCORES = 8
RPC = B * N // NCORES  # 1024 rows per core
RT = RPC // 128        # 8 row tiles per core
KT = D // 128          # 2 contraction tiles
F32 = mybir.dt.float32
AF = mybir.ActivationFunctionType
ALU = mybir.AluOpType
AX = mybir.AxisListType

_CACHE: dict = {}
last_results: list = []  # BassKernelResults of the most recent kernel() call


def _trace_args(tag):
    d = os.environ.get("GT_TRACE_DIR")
    if not d:
        return {}
    td = os.path.join(d, tag)
    os.makedirs(td, exist_ok=True)
    return {"trace": True, "tmpdir": td}


def _bcast(ap_1d, parts, n):
    """DRAM [n] -> broadcast access pattern [parts, n] (0-stride partitions)."""
    return bass.AP(tensor=ap_1d.tensor, offset=ap_1d.offset, ap=[[0, parts], [1, n]])


def _build_stage1():
    nc = bacc.Bacc("TRN2", target_bir_lowering=False, debug=False, num_devices=NCORES)
    hT = nc.dram_tensor("hT", [D, RPC], F32, kind="ExternalInput").ap()
    # wlr[k] = [W_l[k-tile] | W_r[k-tile]] : fused moving operand [128, 512]
    wlr = nc.dram_tensor("wlr", [D, 2 * D], F32, kind="ExternalInput").ap()
    # aflat = [tile(a_l,8)/H | tile(a_r,8)] : [512]
    aflat = nc.dram_tensor("aflat", [2 * D], F32, kind="ExternalInput").ap()
    # [p, jt] layout: row r = jt*128 + p lives at [r % 128, r // 128]
    slb = nc.dram_tensor("slb", [128, RT], F32, kind="ExternalOutput").ap()
    srb = nc.dram_tensor("srb", [128, RT], F32, kind="ExternalOutput").ap()
    sloc = nc.dram_tensor("sloc", [H], F32, kind="ExternalOutput").ap()
    csum = nc.dram_tensor("csum", [H, D], F32, kind="ExternalOutput").ap()

    with tile.TileContext(nc) as tc, ExitStack() as ctx:
        const = ctx.enter_context(tc.tile_pool(name="const", bufs=1))
        work = ctx.enter_context(tc.tile_pool(name="work", bufs=3))
        frp = ctx.enter_context(tc.tile_pool(name="frp", bufs=1))
        ps = ctx.enter_context(tc.tile_pool(name="ps", bufs=5, space="PSUM"))
        ps1 = ctx.enter_context(tc.tile_pool(name="ps1", bufs=1, space="PSUM"))

        sb_hT = []
        sb_wlr = []
        for k in range(KT):
            t = const.tile([128, 2 * D], F32, tag=f"wlr{k}")
            nc.sync.dma_start(t[:], wlr[k * 128:(k + 1) * 128, :])
            sb_wlr.append(t)
        for k in range(KT):
            t = const.tile([128, RPC], F32, tag=f"hT{k}")
            # chunked so the first projection matmul starts ~6us earlier
            for q in range(4):
                cs = slice(q * (RPC // 4), (q + 1) * (RPC // 4))
                nc.sync.dma_start(t[:, cs], hT[k * 128:(k + 1) * 128, cs])
            sb_hT.append(t)
        sb_ab = const.tile([128, 2 * D], F32, tag="ab")
        nc.sync.dma_start(sb_ab[:], _bcast(aflat, 128, 2 * D))
        sb_ones = const.tile([128, 1], F32, tag="ones")
        nc.vector.memset(sb_ones[:], 1.0)

        sb_slc = const.tile([128, RT], F32, tag="slc")
        sb_src = const.tile([128, RT], F32, tag="src")
        pc = ps1.tile([H, D], F32, tag="csum")   # exp-weighted fr sum
        psm = ps1.tile([1, H], F32, tag="sloc")  # exp sums

        ers = []
        for jt in range(RT):
            # [fl | fr] rows for this row tile: psum [128, 512]
            pt = ps.tile([128, 2 * D], F32, tag="mm")
            for k in range(KT):
                nc.tensor.matmul(
                    pt[:],
                    sb_hT[k][:, jt * 128:(jt + 1) * 128],
                    sb_wlr[k][:],
                    start=(k == 0),
                    stop=(k == KT - 1),
                )
            # leaky(x) = 0.01*x + relu(0.99*x)
            rl = work.tile([128, 2 * D], F32, tag="rl")
            nc.scalar.activation(rl[:], pt[:], AF.Relu, scale=1.0 - NEG_SLOPE)
            lk = work.tile([128, 2 * D], F32, tag="lk")
            nc.vector.scalar_tensor_tensor(
                out=lk[:], in0=pt[:], scalar=NEG_SLOPE, in1=rl[:],
                op0=ALU.mult, op1=ALU.add,
            )
            # raw fr rows for the context contraction (ScalarE has slack)
            fr_raw = frp.tile([128, D], F32, tag=f"fr{jt}")
            nc.scalar.copy(fr_raw[:], pt[:, D:2 * D])
            # score dots: sl on Vector (fused mul+reduce), sr on GpSimd
            t3l = work.tile([128, D], F32, tag="t3l")
            nc.vector.tensor_mul(t3l[:], lk[:, 0:D], sb_ab[:, 0:D])
            nc.vector.reduce_sum(out=sb_slc[:, jt:jt + 1], in_=t3l[:], axis=AX.X)
            t3r = work.tile([128, D], F32, tag="t3r")
            nc.vector.tensor_mul(t3r[:], lk[:, D:2 * D], sb_ab[:, D:2 * D])
            srh = work.tile([128, H], F32, tag="srh")
            nc.vector.reduce_sum(
                out=srh[:], in_=t3r[:].rearrange("p (h w) -> p h w", h=H),
                axis=AX.X,
            )
            nc.vector.reduce_sum(out=sb_src[:, jt:jt + 1], in_=srh[:], axis=AX.X)
            # e = exp(sr) (scores are O(1): no max shift; combined on host)
            er = frp.tile([128, H], F32, tag=f"er{jt}")
            nc.scalar.activation(er[:], srh[:], AF.Exp)
            ers.append((er, fr_raw))

        # trailing contractions, emitted after all projections so the PE
        # stream stays dense (HAM warm) instead of stalling on each jt chain
        for jt, (er, fr_raw) in enumerate(ers):
            nc.tensor.matmul(psm[:], sb_ones[:], er[:],
                             start=(jt == 0), stop=(jt == RT - 1))
            nc.tensor.matmul(pc[:], er[:], fr_raw[:],
                             start=(jt == 0), stop=(jt == RT - 1))

        # head-mean for the attn column scores: sr_bar = sum_h srh / H
        sb_srm = const.tile([128, RT], F32, tag="srm")
        nc.scalar.mul(sb_srm[:], sb_src[:], 1.0 / H)

        sb_sloc = work.tile([1, H], F32, tag="slocv")
        nc.vector.tensor_copy(sb_sloc[:], psm[:])
        nc.sync.dma_start(sloc[None, :], sb_sloc[:])
        sb_csum = work.tile([H, D], F32, tag="csumv")
        nc.vector.tensor_copy(sb_csum[:], pc[:])
        nc.sync.dma_start(csum[:], sb_csum[:])
        nc.sync.dma_start(slb[:], sb_slc[:])
        nc.sync.dma_start(srb[:], sb_srm[:])

    nc.compile()
    return nc


def _build_stage2():
    nc = bacc.Bacc("TRN2", target_bir_lowering=False, debug=False, num_devices=NCORES)
    hrows = nc.dram_tensor("hrows", [RPC, D], F32, kind="ExternalInput").ap()
    slcol = nc.dram_tensor("slcol", [128, RT], F32, kind="ExternalInput").ap()
    srbv = nc.dram_tensor("srbv", [N], F32, kind="ExternalInput").ap()
    fhv = nc.dram_tensor("fhv", [D], F32, kind="ExternalInput").ap()
    lng = nc.dram_tensor("lng", [D], F32, kind="ExternalInput").ap()
    lnb = nc.dram_tensor("lnb", [D], F32, kind="ExternalInput").ap()
    attn = nc.dram_tensor("attn", [RPC, N], F32, kind="ExternalOutput").ap()
    hout = nc.dram_tensor("hout", [RPC, D], F32, kind="ExternalOutput").ap()

    NQ = 4          # srb broadcast chunks
    CH = N // NQ    # 1024

    with tile.TileContext(nc) as tc, ExitStack() as ctx:
        const = ctx.enter_context(tc.tile_pool(name="const", bufs=1))
        apool = ctx.enter_context(tc.tile_pool(name="apool", bufs=3))
        hpool = ctx.enter_context(tc.tile_pool(name="hpool", bufs=3))
        spool = ctx.enter_context(tc.tile_pool(name="spool", bufs=4))

        sb_sl = const.tile([128, RT], F32, tag="sl")
        nc.sync.dma_start(sb_sl[:], slcol[:])
        sb_sr = const.tile([128, N], F32, tag="sr")
        for q in range(NQ):
            nc.sync.dma_start(
                sb_sr[:, q * CH:(q + 1) * CH],
                _bcast(srbv[q * CH:(q + 1) * CH], 128, CH),
            )
        sb_fh = const.tile([128, D], F32, tag="fh")
        nc.sync.dma_start(sb_fh[:], _bcast(fhv, 128, D))
        sb_g = const.tile([128, D], F32, tag="g")
        nc.sync.dma_start(sb_g[:], _bcast(lng, 128, D))
        sb_b = const.tile([128, D], F32, tag="b")
        nc.sync.dma_start(sb_b[:], _bcast(lnb, 128, D))
        sb_eps = const.tile([128, 1], F32, tag="eps")
        nc.vector.memset(sb_eps[:], LN_EPS)

        for t in range(RT):
            rs_ = slice(t * 128, (t + 1) * 128)
            at = apool.tile([128, N], F32, tag="at")
            # outer sum, halves on different engines; DMA each half when ready,
            # alternating issue engines to spread DGE queues
            nc.vector.tensor_scalar_add(at[:, 0:N // 2], sb_sr[:, 0:N // 2], sb_sl[:, t:t + 1])
            nc.sync.dma_start(attn[rs_, 0:N // 2], at[:, 0:N // 2])
            nc.scalar.activation(at[:, N // 2:N], sb_sr[:, N // 2:N], AF.Identity, bias=sb_sl[:, t:t + 1])
            nc.scalar.dma_start(attn[rs_, N // 2:N], at[:, N // 2:N])

            ht = hpool.tile([128, D], F32, tag="ht")
            nc.sync.dma_start(ht[:], hrows[rs_, :])
            xt = hpool.tile([128, D], F32, tag="xt")
            nc.vector.tensor_add(xt[:], ht[:], sb_fh[:])
            st = spool.tile([128, 6], F32, tag="st")
            nc.vector.bn_stats(out=st[:], in_=xt[:])
            mv = spool.tile([128, 2], F32, tag="mv")
            nc.vector.bn_aggr(out=mv[:], in_=st[:])
            rstd = spool.tile([128, 1], F32, tag="rstd")
            nc.scalar.activation(rstd[:], mv[:, 1:2], AF.Sqrt, bias=sb_eps[:])
            nc.vector.reciprocal(rstd[:], rstd[:])
            yt = hpool.tile([128, D], F32, tag="yt")
            nc.vector.tensor_scalar(
                out=yt[:], in0=xt[:],
                scalar1=mv[:, 0:1], scalar2=rstd[:],
                op0=ALU.subtract, op1=ALU.mult,
            )
            nc.vector.tensor_mul(yt[:], yt[:], sb_g[:])
            nc.vector.tensor_add(yt[:], yt[:], sb_b[:])
            nc.sync.dma_start(hout[rs_, :], yt[:])

    nc.compile()
    return nc


def _get_programs():
    if "s1" not in _CACHE:
        _CACHE["s1"] = _build_stage1()
        _CACHE["s2"] = _build_stage2()
    return _CACHE["s1"], _CACHE["s2"]


def kernel(h, W_l, W_r, a_l, a_r, W_final, ln_g, ln_b):
    global last_results
    h = np.ascontiguousarray(np.asarray(h, np.float32))
    W_l = np.asarray(W_l, np.float32)
    W_r = np.asarray(W_r, np.float32)
    a_l = np.asarray(a_l, np.float32)
    a_r = np.asarray(a_r, np.float32)
    W_final = np.asarray(W_final, np.float32)
    ln_g = np.asarray(ln_g, np.float32)
    ln_b = np.asarray(ln_b, np.float32)

    nc1, nc2 = _get_programs()
    hf = h.reshape(B * N, D)

    wlr = np.concatenate([W_l, W_r], axis=1)
    aflat = np.concatenate([np.tile(a_l, H) / H, np.tile(a_r, H)]).astype(np.float32)

    in1 = []
    for c in range(NCORES):
        rows = hf[c * RPC:(c + 1) * RPC]
        in1.append({"hT": np.ascontiguousarray(rows.T), "wlr": wlr, "aflat": aflat})
    res1 = run_bass_kernel_spmd(nc1, in1, list(range(NCORES)), **_trace_args("s1"))
    r1 = res1.results

    # slb/srb are [128, RT] per core with row r at [r % 128, r // 128]
    srbf = np.concatenate([r1[c]["srb"].T.reshape(-1) for c in range(NCORES)])
    sloc = np.stack([r1[c]["sloc"] for c in range(NCORES)])   # [8, H]
    csum = np.stack([r1[c]["csum"] for c in range(NCORES)])   # [8, H, D]

    # global softmax combine + (degenerate, one row per batch) W_final matmul
    fh = np.zeros((B, D), np.float32)
    for b in range(B):
        cs = slice(b * (NCORES // B), (b + 1) * (NCORES // B))
        S = sloc[cs].sum(axis=0)                  # [H]
        Cs = csum[cs].sum(axis=0)                 # [H, D]
        cvec = np.stack([Cs[hh, HD * hh:HD * (hh + 1)] / S[hh] for hh in range(H)])
        fh[b] = cvec.reshape(D).astype(np.float32) @ W_final

    sr_bar = srbf.reshape(B, N)

    in2 = []
    for c in range(NCORES):
        b = c // (NCORES // B)
        in2.append({
            "hrows": hf[c * RPC:(c + 1) * RPC],
            "slcol": r1[c]["slb"],   # already [128, RT] = [p, jt]
            "srbv": sr_bar[b],
            "fhv": fh[b],
            "lng": ln_g,
            "lnb": ln_b,
        })
    res2 = run_bass_kernel_spmd(nc2, in2, list(range(NCORES)), **_trace_args("s2"))
    r2 = res2.results
    last_results = [res1, res2]

    attn = np.concatenate([r2[c]["attn"] for c in range(NCORES)]).reshape(B, N, N)
    hout = np.concatenate([r2[c]["hout"] for c in range(NCORES)]).reshape(B, N, D)
    return hout, attn
